# revision 77
# baseline (speedup 1.0000x reference)
"""Trainium2 Bass kernel for nn_Biaffine (B=4, S=512, D=512, R=64).

Math: the reference computes
    left = einsum('bxi,irj,byj->bxyr', hf, U1, hb)
    out  = mean_y(left + rf[:, :, None] + rb[:, None] + bias)
The mean over y commutes with everything:
    mean_y(left)[b,x,r] = sum_ij hf[b,x,i] U1[i,r,j] hbbar[b,j],
    hbbar = mean_y(hb).
So out[b,x,r] = sum_i hf[b,x,i] * (V[b,i,r] + U2a[i,r]) + rbbar[b,r] + bias[r]
with V[b,i,r] = sum_j U1[i,r,j] hbbar[b,j], rbbar = hbbar @ U2b.

Sharding: tensor-parallel over r (dep_vec_dim): core c owns r in [8c, 8c+8).
PE operands are host-cast to low precision (tolerance 2e-2, measured ~1e-3
in fp32): hf/hb in bf16, and U1 — the dominant DMA traffic — in fp8 e3m4
(4 mantissa bits). fp8 needs power-of-2 scale folding to stay in e3m4's
±15.5 normal range: U1 is pre-scaled by 1024 (|U1| <= 0.0134), the raw
y-sum of hb by 1/8, so V comes out of PSUM at 2^16x and is descaled in the
PSUM->SBUF tensor_scalar copy. The 1/S mean factor folds into the same
scales. Each core reads the full hb (2MB bf16) and takes the y-mean
on-device with DVE reduces — no collective, cores run fully independently.
DMA is ~6.3MB/core (~18us at the ~343GB/s bus); V matmuls stream per
j-chunk as U1 lands and hft arrives just-in-time for the tail matmuls.
"""

import os
import sys

import numpy as np

try:
    import concourse.bass as bass  # noqa: F401
except ImportError:  # pragma: no cover
    sys.path.insert(0, "/opt/trn_rl_repo")

B, S, D, R = 4, 512, 512, 64
NCORES = 8
RB = R // NCORES  # 8 r's per core
P = 128
JC = D // P  # 4 j-chunks
IC = D // P  # 4 i-chunks

TRACE = os.environ.get("BASS_KERNEL_TRACE", "0") == "1"

# power-of-2 scale folding for the fp8 (e3m4, range +-15.5) V matmul chain:
#   u1t = U1^T * U1_SCALE            (|U1| <= 0.0134 -> +-13.7)
#   hbbarT8 = (sum_y hb) * HBSUM_SCALE  (std 22.6 -> 2.8, max ~10)
#   PSUM V = true V * U1_SCALE * HBSUM_SCALE * S  = V * 2^16
#   u2b8 = U2b * U2B_SCALE; PSUM rbbar = rbbar * S * HBSUM_SCALE * U2B_SCALE
U1_SCALE = 1024.0
HBSUM_SCALE = 1.0 / 8.0
U2B_SCALE = 64.0
VQ_DESCALE = 1.0 / (U1_SCALE * HBSUM_SCALE * S)  # 2^-16
RB_DESCALE = 1.0 / (HBSUM_SCALE * U2B_SCALE * S)  # 2^-12

_NC_CACHE = {}


def _build_nc(n_repeat=1):
    import concourse.bacc as bacc
    import concourse.mybir as mybir
    import concourse.tile as tile
    from concourse.masks import make_identity
    fp32 = mybir.dt.float32
    bf16 = mybir.dt.bfloat16
    fp8 = mybir.dt.float8e3  # e3m4: DVE-only input (hb), best 8-bit mantissa
    fp8m = mybir.dt.float8e4  # e4m3: PE matmul operands, DoubleRow-capable

    nc = bacc.Bacc("TRN2", target_bir_lowering=False, debug=False, num_devices=NCORES)

    hft_d = nc.dram_tensor("hft", [B, P, IC, S], bf16, kind="ExternalInput")
    hbt_d = nc.dram_tensor("hbt", [D, B, S], fp8, kind="ExternalInput")
    u1t_d = nc.dram_tensor("u1t", [D, RB, D], fp8, kind="ExternalInput")
    u2a_d = nc.dram_tensor("u2a", [P, IC, RB], bf16, kind="ExternalInput")
    u2b_d = nc.dram_tensor("u2b", [P, JC, RB], fp8, kind="ExternalInput")
    bias_d = nc.dram_tensor("biasr", [1, RB], bf16, kind="ExternalInput")
    out_d = nc.dram_tensor("out", [B, RB, S], bf16, kind="ExternalOutput")

    with tile.TileContext(nc) as tc:
        with (
            tc.tile_pool(name="const", bufs=1) as cpool,
            tc.tile_pool(name="data", bufs=1) as dpool,
            tc.tile_pool(name="psum", bufs=8, space="PSUM") as ppool,
        ):
            identity_sq = cpool.tile([100, 100], bf16, tag="identity_sq")
            make_identity(nc, identity_sq)
            ones1 = cpool.tile([1, S], bf16, tag="ones1")
            nc.vector.memset(ones1, 1.0)

            for _rep in range(n_repeat):
                _emit_body(
                    nc, dpool, ppool, fp32, bf16, fp8, fp8m, ones1,
                    identity_sq, hft_d, hbt_d, u1t_d, u2a_d, u2b_d, bias_d,
                    out_d,
                )

    nc.compile()
    return nc


def _emit_body(
    nc, dpool, ppool, fp32, bf16, fp8, fp8m, ones1, identity_sq,
    hft_d, hbt_d, u1t_d, u2a_d, u2b_d, bias_d, out_d,
):
    import concourse.mybir as mybir

    u2a_sb = dpool.tile([P, IC, RB], bf16, tag="u2a_sb", bufs=2)
    u2b_sb = dpool.tile([P, JC, RB], fp8, tag="u2b_sb", bufs=2)
    bias_sb = dpool.tile([1, RB], bf16, tag="bias_sb", bufs=2)
    hbbarT_f = dpool.tile([P, JC, B], fp32, tag="hbbarT_f", bufs=2)
    hbbarT = dpool.tile([P, JC, B], fp8, tag="hbbarT", bufs=2)
    vass = dpool.tile([P, IC, B, RB], bf16, tag="vass", bufs=2)

    # --- small inputs ride the scalar queue so they don't delay hbt0's
    # DGE on sync (the first V-chain dependency); everything else streams
    # on the sync queue in dependency order: hb chunks (the y-mean gates
    # every V matmul), u1 r-major — each 256KB r-chunk completes V for
    # that r outright so per-r postprocessing streams — then hft b-major
    # half-chunks so the tail out-matmuls fire as each piece lands.
    nc.scalar.dma_start(out=u2a_sb, in_=u2a_d.ap())
    nc.scalar.dma_start(out=u2b_sb, in_=u2b_d.ap())
    nc.scalar.dma_start(out=bias_sb, in_=bias_d.ap())

    hbt_tiles, u1_tiles = [], []
    for jc in range(JC):
        hbt_t = dpool.tile([P, B, S], fp8, tag=f"hbt{jc}", bufs=2)
        nc.sync.dma_start(out=hbt_t, in_=hbt_d.ap()[jc * P : (jc + 1) * P])
        hbt_tiles.append(hbt_t)
    for jc in range(JC):
        u1t_t = dpool.tile([P, RB, D], fp8, tag=f"u1_{jc}")
        nc.sync.dma_start(out=u1t_t, in_=u1t_d.ap()[jc * P : (jc + 1) * P])
        u1_tiles.append(u1t_t)
    hft_tiles = []
    for b in range(B):
        hft_t = dpool.tile([P, IC, S], bf16, tag=f"hft{b}", bufs=2)
        nc.sync.dma_start(out=hft_t, in_=hft_d.ap()[b])
        hft_tiles.append(hft_t)

    # --- hbbar: DVE reduces pipelined behind the hb chunk arrivals; the
    # fp8 cast carries the HBSUM_SCALE fold; rbbar accumulates along the
    # same stream. V(r, jc) only waits on the hbbar jc-slice it reads.
    ps_rb = ppool.tile([P, 512], fp32, tag="ps")
    for jc in range(JC):
        for b in range(B):
            nc.vector.reduce_sum(
                hbbarT_f[:, jc, b : b + 1],
                hbt_tiles[jc][:, b, :],
                axis=mybir.AxisListType.X,
            )
        nc.vector.tensor_scalar_mul(
            hbbarT[:, jc, :],
            hbbarT_f[:, jc, :],
            HBSUM_SCALE,
        )
        nc.tensor.matmul(
            ps_rb[:B, :RB],
            hbbarT[:, jc, :],
            u2b_sb[:, jc, :],
            start=(jc == 0),
            stop=False,
        )

    # --- V[b, i] per r, streamed r-major: hbbarT stationary (LDW = 4
    # cols), the r-chunk of U1 streams as the N=512 moving operand, all
    # four j-chunks back-to-back. Four r's share one PSUM tile at base
    # partitions {0,32,64,96} (legal tile_position[1] for M=4), so the
    # [b, i] -> [i, b] PE transposes drop from 32 to 8.
    ps_qs = []
    for _rq in range(RB // 4):
        ps_q = ppool.tile([P, 512], fp32, tag="ps")
        ps_qs.append(ps_q)
    for jc in range(JC):
        for r in range(RB):
            rq, k = divmod(r, 4)
            nc.tensor.matmul(
                ps_qs[rq][k * 32 : k * 32 + B, :D],
                hbbarT[:, jc, :],
                u1_tiles[jc][:, r, :],
                start=(jc == 0),
                stop=(jc == JC - 1),
                tile_position=(0, k * 32),
            )

    # --- bias joins rbbar via a K=1 ones-matmul (pre-scaled on host to
    # match ps_rb's folded scale) ---
    nc.tensor.matmul(
        ps_rb[:B, :RB], ones1[:1, :B], bias_sb, start=False, stop=True
    )
    rbb = dpool.tile([B, RB], bf16, tag="rbb", bufs=2)
    nc.vector.tensor_scalar_mul(rbb, ps_rb[:B, :RB], RB_DESCALE)
    # transpose to [r, b] so (rbbar+bias) can be added to the output
    # tiles as a per-partition broadcast during the PSUM->SBUF copy
    ps_rbt = ppool.tile([P, 512], bf16, tag="ps")
    nc.tensor.transpose(ps_rbt[:RB, :B], rbb, identity_sq[:B, :B])
    rbbT = dpool.tile([RB, B], fp32, tag="rbbT", bufs=2)
    nc.vector.tensor_copy(out=rbbT, in_=ps_rbt[:RB, :B])

    # --- vass[i, ic, b, r] = V^T + U2a, cast to bf16 for the out matmuls
    for rq in range(RB // 4):
        vq = dpool.tile([100, D], bf16, tag="vq", bufs=2)
        nc.vector.tensor_scalar_mul(vq, ps_qs[rq][:100, :D], VQ_DESCALE)
        for ic in range(IC):
            ps_t = ppool.tile([P, 512], bf16, tag="ps")
            nc.tensor.transpose(
                ps_t[:P, :100], vq[:, ic * P : (ic + 1) * P], identity_sq
            )
            # one strided add moves all 4 r's: ps_t cols (k*32 + b),
            # viewed [p, k, b] -> [p, b, k], into vass[:, ic, b, r]
            nc.vector.tensor_tensor(
                out=vass[:, ic, :, rq * 4 : (rq + 1) * 4],
                in0=ps_t[:, :128]
                .rearrange("p (k c) -> p k c", c=32)[:, :, :B]
                .rearrange("p k b -> p b k"),
                in1=u2a_sb[:, ic, None, rq * 4 : (rq + 1) * 4].to_broadcast(
                    (P, B, 4)
                ),
                op=mybir.AluOpType.add,
            )

    # --- out[r, x] per b: contract i; rbbar+bias added on the PSUM read ---
    for b in range(B):
        ps_o = ppool.tile([P, 512], fp32, tag="ps")
        for ic in range(IC):
            nc.tensor.matmul(
                ps_o[:RB, :S],
                vass[:, ic, b, :],
                hft_tiles[b][:, ic, :],
                start=(ic == 0),
                stop=(ic == IC - 1),
            )
        out_sb_b = dpool.tile([RB, S], bf16, tag=f"out{b}", bufs=2)
        nc.vector.tensor_tensor(
            out=out_sb_b,
            in0=ps_o[:RB, :S],
            in1=rbbT[:, b : b + 1].to_broadcast((RB, S)),
            op=mybir.AluOpType.add,
        )
        nc.scalar.dma_start(out=out_d.ap()[b], in_=out_sb_b)


def _get_nc(n_repeat=1):
    if n_repeat not in _NC_CACHE:
        _NC_CACHE[n_repeat] = _build_nc(n_repeat)
    return _NC_CACHE[n_repeat]


def _prep_inputs(h_forward, h_backward, U_1, U_2, bias):
    import ml_dtypes

    bf16 = ml_dtypes.bfloat16
    hf = np.asarray(h_forward, dtype=np.float32)
    hb = np.asarray(h_backward, dtype=np.float32)
    u1 = np.asarray(U_1, dtype=np.float32)
    u2 = np.asarray(U_2, dtype=np.float32)
    bz = np.asarray(bias, dtype=np.float32)

    # hft host-packed to [B, i%P, ichunk, x] so each DMA partition row is
    # contiguous; bf16 for the PE
    hft = np.ascontiguousarray(
        hf.transpose(0, 2, 1).reshape(B, IC, P, S).transpose(0, 2, 1, 3)
    ).astype(bf16)
    # hbt [j, b, y]: full hb per core in fp8 e3m4 (|hb| < 8 fits the +-15.5
    # range raw); the y-mean happens on-device
    hbt = np.ascontiguousarray(hb.transpose(2, 0, 1)).astype(
        ml_dtypes.float8_e3m4
    )

    fp8 = ml_dtypes.float8_e3m4
    in_maps = []
    for c in range(NCORES):
        rs = slice(c * RB, (c + 1) * RB)
        # fp8 e4m3 (DoubleRow-capable) with the U1_SCALE fold (see module
        # scale constants); the 1/S mean factor is carried by VQ_DESCALE on
        # the PSUM read. r-major layout [r, j%P, jchunk, i]: one contiguous
        # 256KB DMA per r
        u1t_c = np.ascontiguousarray(
            u1[:, rs, :].transpose(2, 1, 0) * np.float32(U1_SCALE)
        ).astype(fp8)  # [j, r, i]
        # u2a pre-packed [d%P, dchunk, RB] (bf16, raw); u2b [d%P, dchunk, RB]
        # (fp8, *U2B_SCALE; RB_DESCALE folds it back with 1/S)
        u2a_c = np.ascontiguousarray(
            u2[:D, rs].reshape(IC, P, RB).transpose(1, 0, 2)
        ).astype(bf16)
        u2b_c = np.ascontiguousarray(
            u2[D:, rs].reshape(JC, P, RB).transpose(1, 0, 2)
            * np.float32(U2B_SCALE)
        ).astype(fp8)
        bias_c = np.ascontiguousarray(
            bz[rs].reshape(1, RB) / np.float32(RB_DESCALE)
        ).astype(bf16)
        in_maps.append(
            {
                "hft": hft,
                "hbt": hbt,
                "u1t": u1t_c,
                "u2a": u2a_c,
                "u2b": u2b_c,
                "biasr": bias_c,
            }
        )
    return in_maps


def _get_exec():
    """One jitted sharded executable, cached for the process lifetime.

    Repeated kernel() calls reuse it — re-jitting a second executable in the
    same process has been observed to wedge the NRT
    (NRT_EXEC_UNIT_UNRECOVERABLE), while re-executing one executable is solid.
    """
    if "exec" in _EXEC_CACHE:
        return _EXEC_CACHE["exec"]

    import jax
    from jax.sharding import Mesh, PartitionSpec

    import warnings

    with warnings.catch_warnings():
        warnings.simplefilter("ignore")
        from jax.experimental.shard_map import shard_map

    from concourse import mybir
    from concourse.bass2jax import (
        _bass_exec_p,
        install_neuronx_cc_hook,
        partition_id_tensor,
    )

    install_neuronx_cc_hook()
    nc = _get_nc()
    partition_name = nc.partition_id_tensor.name if nc.partition_id_tensor else None
    in_names, out_names, out_avals = [], [], []
    for alloc in nc.m.functions[0].allocations:
        if not isinstance(alloc, mybir.MemoryLocationSet):
            continue
        name = alloc.memorylocations[0].name
        if alloc.kind == "ExternalInput":
            if name != partition_name:
                in_names.append(name)
        elif alloc.kind == "ExternalOutput":
            out_names.append(name)
            out_avals.append(
                jax.core.ShapedArray(tuple(alloc.tensor_shape), mybir.dt.np(alloc.dtype))
            )
    all_names = in_names + out_names
    if partition_name is not None:
        all_names = all_names + [partition_name]

    def _body(*args):
        operands = list(args)
        if partition_name is not None:
            operands.append(partition_id_tensor())
        return tuple(
            _bass_exec_p.bind(
                *operands,
                out_avals=tuple(out_avals),
                in_names=tuple(all_names),
                out_names=tuple(out_names),
                lowering_input_output_aliases=(),
                sim_require_finite=True,
                sim_require_nnan=True,
                nc=nc,
            )
        )

    devices = jax.devices()[:NCORES]
    mesh = Mesh(np.asarray(devices), ("core",))
    n_args = len(in_names) + len(out_avals)
    fn = jax.jit(
        shard_map(
            _body,
            mesh=mesh,
            in_specs=(PartitionSpec("core"),) * n_args,
            out_specs=(PartitionSpec("core"),) * len(out_names),
            check_rep=False,
        ),
        keep_unused=True,
    )
    sh = jax.sharding.NamedSharding(mesh, PartitionSpec("core"))
    _EXEC_CACHE["exec"] = (fn, sh, in_names, out_names, out_avals)
    return _EXEC_CACHE["exec"]


_EXEC_CACHE = {}


def kernel(h_forward, h_backward, U_1, U_2, bias):
    import jax

    fn, sh, in_names, out_names, out_avals = _get_exec()
    in_maps = _prep_inputs(h_forward, h_backward, U_1, U_2, bias)
    args = [
        jax.device_put(
            np.concatenate([in_maps[c][name] for c in range(NCORES)], axis=0), sh
        )
        for name in in_names
    ]
    for av in out_avals:
        args.append(
            jax.device_put(
                np.zeros((NCORES * av.shape[0], *av.shape[1:]), av.dtype), sh
            )
        )
    out_arrs = fn(*args)
    oi = out_names.index("out")
    full = (
        np.asarray(out_arrs[oi])
        .astype(np.float32)
        .reshape(NCORES, B, RB, S)  # [core, B, RB, S]
    )
    out = np.concatenate(list(full), axis=1)  # [B, R, S]
    return np.ascontiguousarray(out.transpose(0, 2, 1))  # [B, S, R]
